# revision 4
# baseline (speedup 1.0000x reference)
"""Distributed Trainium2 kernel for the 4-layer single-head causal-attention
stack (returns mean attention weights over layers).

Sharding: sequence-parallel over the 2048 mentions. 16 row-tiles of 128;
core c owns tiles {c, 15-c} so the causal-attention column counts
(128*(c+1) + 128*(16-c) == 17*128) are identical on every core -> one
uniform SPMD program. Per layer each core projects K,V for its 256 rows,
all-gathers K,V across the 8 cores (one collective), projects Q while the
gather runs, then computes masked scores, softmax, W@V and the output
projection in bf16 with f32 PSUM accumulation. Layer 3 only needs Q,K
(its attention output is never consumed). Attention weights accumulate in
f32 and are written out as the mean over the 4 layers.
"""

import numpy as np
import ml_dtypes

N, E, L, NCORES = 2048, 1024, 4, 8
EC = E // 128          # 8 contraction chunks of 128
MT = 256               # mention rows per core
SCALE = 1.0 / np.sqrt(np.float32(E))
KV_K_ELEMS = E * MT            # k block: [1024, 256] (feature-major)
KV_V_ELEMS = MT * E            # v block: [256, 1024] (row-major natural)
KV_ELEMS = KV_K_ELEMS + KV_V_ELEMS
NEG = -1e30

BF16 = ml_dtypes.bfloat16

_RUNNER = None


def _build_nc():
    import concourse.mybir as mybir
    import concourse.tile as tile
    from concourse import bacc

    f32 = mybir.dt.float32
    bf16 = mybir.dt.bfloat16

    nc = bacc.Bacc("TRN2", target_bir_lowering=False, debug=False,
                   num_devices=NCORES)

    xt_p = nc.declare_dram_parameter("xt", [E, MT], bf16, isOutput=False)
    wqkvt_p = nc.declare_dram_parameter("wqkvt", [L * E, 3 * E], bf16, isOutput=False)
    wot_p = nc.declare_dram_parameter("wot", [L * E, E], bf16, isOutput=False)
    bqkv_p = nc.declare_dram_parameter("bqkv", [L * 3 * E], f32, isOutput=False)
    bo_p = nc.declare_dram_parameter("bo", [L * E], f32, isOutput=False)
    maska_p = nc.declare_dram_parameter("maska", [128, 1024], f32, isOutput=False)
    maskb_p = nc.declare_dram_parameter("maskb", [128, 2048], f32, isOutput=False)
    ident_p = nc.declare_dram_parameter("ident", [128, 128], bf16, isOutput=False)
    out_p = nc.declare_dram_parameter("out", [MT, N], f32, isOutput=True)

    AOP = mybir.AluOpType
    AF = mybir.ActivationFunctionType

    from contextlib import ExitStack

    with tile.TileContext(nc) as tc:
        with ExitStack() as stack:
            ep = lambda **kw: stack.enter_context(tc.tile_pool(**kw))
            dram = ep(name="dram", bufs=2, space="DRAM")
            consts = ep(name="consts", bufs=1)
            px = ep(name="px", bufs=2)
            pq = ep(name="pq", bufs=2)
            pktf = ep(name="pktf", bufs=1)
            pvf = ep(name="pvf", bufs=1)
            pscore = ep(name="pscore", bufs=1)
            pw = ep(name="pw", bufs=1)
            pwvt = ep(name="pwvt", bufs=2)
            pacc = ep(name="pacc", bufs=1)
            pwqk = ep(name="pwqk", bufs=4)
            pwv = ep(name="pwv", bufs=2)
            pwo = ep(name="pwo", bufs=4)
            pstage = ep(name="pstage", bufs=4)
            pbias = ep(name="pbias", bufs=2)
            pstats = ep(name="pstats", bufs=4)
            psmm = ep(name="psmm", bufs=2, space="PSUM")
            pssc = ep(name="pssc", bufs=2, space="PSUM")
            pssm = ep(name="pssm", bufs=2, space="PSUM")

            ident = consts.tile([128, 128], bf16)
            nc.sync.dma_start(ident[:], ident_p[:, :])
            maska = consts.tile([128, 1024], f32)
            nc.sync.dma_start(maska[:], maska_p[:, :])
            maskb = consts.tile([128, 2048], f32)
            nc.sync.dma_start(maskb[:], maskb_p[:, :])
            zeros = consts.tile([128, 1024], f32)
            nc.vector.memset(zeros[:], 0.0)
            acc_a = pacc.tile([128, 1024], f32, tag="acca")
            nc.vector.memset(acc_a[:], 0.0)
            acc_b = pacc.tile([128, 2048], f32, tag="accb")
            nc.vector.memset(acc_b[:], 0.0)

            xt = px.tile([128, EC, MT], bf16, tag="xt")
            nc.sync.dma_start(
                xt[:], xt_p.ap().rearrange("(c p) m -> p c m", p=128))

            for li in range(L):
                last = li == L - 1
                wrow = li * E  # weight row offset for this layer

                bq = pbias.tile([128, 24], f32, tag="bq")
                nc.sync.dma_start(
                    bq[:],
                    bqkv_p.ap()[li * 3 * E:(li + 1) * 3 * E]
                    .rearrange("(c p) -> p c", p=128))
                if not last:
                    bo_t = pbias.tile([128, 8], f32, tag="bo")
                    nc.sync.dma_start(
                        bo_t[:],
                        bo_p.ap()[li * E:(li + 1) * E]
                        .rearrange("(c p) -> p c", p=128))

                kv_s = dram.tile([KV_ELEMS], bf16, tag="kvs")
                kv_d = dram.tile([KV_ELEMS * NCORES], bf16, tag="kvd")

                # ---- K projection (features 1024:2048 -> f_tiles 8..15) ----
                for ft in range(8, 16):
                    wt = pwqk.tile([128, EC, 128], bf16, tag="wqk")
                    nc.sync.dma_start(
                        wt[:],
                        wqkvt_p.ap()[wrow:wrow + E, 128 * ft:128 * (ft + 1)]
                        .rearrange("(c p) f -> p c f", p=128))
                    ps = psmm.tile([128, MT], f32, tag="mm")
                    for ec in range(EC):
                        nc.tensor.matmul(ps[:], wt[:, ec, :], xt[:, ec, :],
                                         start=(ec == 0), stop=(ec == EC - 1))
                    kst = pstage.tile([128, MT], bf16, tag="kst")
                    nc.scalar.activation(kst[:], ps[:], AF.Identity,
                                         bias=bq[:, ft:ft + 1])
                    kt = ft - 8
                    nc.sync.dma_start(
                        kv_s[kt * 128 * MT:(kt + 1) * 128 * MT]
                        .rearrange("(p m) -> p m", p=128),
                        kst[:])

                # ---- V projection (natural layout [m, e]) ----
                if not last:
                    for s in range(2):
                        wvt_w = pwv.tile([128, EC, 512], bf16, tag="wv")
                        nc.sync.dma_start(
                            wvt_w[:],
                            wqkvt_p.ap()[wrow:wrow + E,
                                         2048 + 512 * s:2048 + 512 * (s + 1)]
                            .rearrange("(c p) f -> p c f", p=128))
                        for mt in range(2):
                            ps = psmm.tile([128, 512], f32, tag="mm")
                            for ec in range(EC):
                                nc.tensor.matmul(
                                    ps[:], xt[:, ec, 128 * mt:128 * (mt + 1)],
                                    wvt_w[:, ec, :],
                                    start=(ec == 0), stop=(ec == EC - 1))
                            vst = pstage.tile([128, 512], bf16, tag="vst")
                            nc.scalar.copy(vst[:], ps[:])
                            base = KV_K_ELEMS + mt * 128 * E
                            nc.sync.dma_start(
                                kv_s[base:base + 128 * E]
                                .rearrange("(p e) -> p e", p=128)
                                [:, 512 * s:512 * (s + 1)],
                                vst[:])

                # ---- all-gather K,V ----
                nc.gpsimd.collective_compute(
                    "AllGather", AOP.bypass,
                    replica_groups=[list(range(NCORES))],
                    ins=[kv_s[:].opt()],
                    outs=[kv_d[:].opt()],
                )

                # ---- Q projection (features 0:1024, pre-scaled weights) ----
                qt = pq.tile([128, EC, MT], bf16, tag="qt")
                for ft in range(8):
                    wt = pwqk.tile([128, EC, 128], bf16, tag="wqk")
                    nc.sync.dma_start(
                        wt[:],
                        wqkvt_p.ap()[wrow:wrow + E, 128 * ft:128 * (ft + 1)]
                        .rearrange("(c p) f -> p c f", p=128))
                    ps = psmm.tile([128, MT], f32, tag="mm")
                    for ec in range(EC):
                        nc.tensor.matmul(ps[:], wt[:, ec, :], xt[:, ec, :],
                                         start=(ec == 0), stop=(ec == EC - 1))
                    nc.scalar.activation(qt[:, ft, :], ps[:], AF.Identity,
                                         bias=bq[:, ft:ft + 1])

                # ---- unpack gathered K into [128, ec, 2048] ----
                ktf = pktf.tile([128, EC, N], bf16, tag="ktf")
                for r in range(NCORES):
                    src = (kv_d[r * KV_ELEMS:r * KV_ELEMS + KV_K_ELEMS]
                           .rearrange("(c p m) -> p c m", p=128, m=MT))
                    for h, col in ((0, 128 * r), (1, 128 * (15 - r))):
                        nc.sync.dma_start(
                            ktf[:, :, col:col + 128],
                            src[:, :, 128 * h:128 * (h + 1)])

                # ---- unpack gathered V into natural [n, e] tiles ----
                if not last:
                    vf = pvf.tile([128, 16, E], bf16, tag="vf")
                    for t in range(16):
                        r = t if t < 8 else 15 - t
                        mt = 0 if t < 8 else 1
                        base = r * KV_ELEMS + KV_K_ELEMS + mt * 128 * E
                        nc.sync.dma_start(
                            vf[:, t, :],
                            kv_d[base:base + 128 * E]
                            .rearrange("(p e) -> p e", p=128))

                # ---- scores + softmax + accumulate, per m-tile ----
                w_a = pw.tile([128, 1024], bf16, tag="wa")
                w_b = pw.tile([128, 2048], bf16, tag="wb")
                for mt, width, mask_t, w_t, acc_t, stag in (
                    (0, 1024, maska, w_a, acc_a, "a"),
                    (1, 2048, maskb, w_b, acc_b, "b"),
                ):
                    scores = pscore.tile([128, width], f32, tag=f"sc{stag}")
                    for ns in range(width // 512):
                        ps = pssc.tile([128, 512], f32, tag="sc")
                        for ec in range(EC):
                            nc.tensor.matmul(
                                ps[:], qt[:, ec, 128 * mt:128 * (mt + 1)],
                                ktf[:, ec, 512 * ns:512 * (ns + 1)],
                                start=(ec == 0), stop=(ec == EC - 1))
                        nc.vector.scalar_tensor_tensor(
                            out=scores[:, 512 * ns:512 * (ns + 1)],
                            in0=ps[:], scalar=1.0,
                            in1=mask_t[:, 512 * ns:512 * (ns + 1)],
                            op0=AOP.mult, op1=AOP.add)
                    expv = pscore.tile([128, width], bf16, tag=f"ex{stag}")
                    rowsum = pstats.tile([128, 1], f32, tag="rs")
                    nc.scalar.activation(expv[:], scores[:], AF.Exp,
                                         accum_out=rowsum[:])
                    recip = pstats.tile([128, 1], f32, tag="rc")
                    nc.vector.reciprocal(recip[:], rowsum[:])
                    nc.vector.tensor_scalar_mul(w_t[:], expv[:], recip[:])
                    nc.vector.scalar_tensor_tensor(
                        out=acc_t[:], in0=w_t[:], scalar=1.0, in1=acc_t[:],
                        op0=AOP.mult, op1=AOP.add)

                if last:
                    continue

                # ---- transpose W tiles (PE transpose via identity) ----
                # slot layout: 2t = tile-A chunk t (t<8), 2t+1 = tile-B chunk t
                wtr = pw.tile([128, 32, 128], bf16, tag="wt")
                for mt, w_t, nch in ((0, w_a, 8), (1, w_b, 16)):
                    for t in range(nch):
                        pst = pssm.tile([128, 128], bf16, tag="smt")
                        nc.tensor.transpose(pst[:], w_t[:, 128 * t:128 * (t + 1)],
                                            ident[:])
                        slot = 2 * t + mt
                        nc.vector.tensor_copy(out=wtr[:, slot, :], in_=pst[:])

                # ---- W @ V  (wvT[e', m] = sum_n v[n, e'] * wT[n, m]) ----
                wvt = pwvt.tile([128, EC, MT], bf16, tag="wvt")
                for ep in range(EC):
                    ps_a = pssm.tile([128, 128], f32, tag="sm")
                    for t in range(8):
                        nc.tensor.matmul(
                            ps_a[:], vf[:, t, 128 * ep:128 * (ep + 1)],
                            wtr[:, 2 * t, :],
                            start=(t == 0), stop=(t == 7))
                    ps_b = pssm.tile([128, 128], f32, tag="sm")
                    for t in range(16):
                        nc.tensor.matmul(
                            ps_b[:], vf[:, t, 128 * ep:128 * (ep + 1)],
                            wtr[:, 2 * t + 1, :],
                            start=(t == 0), stop=(t == 15))
                    nc.scalar.copy(wvt[:, ep, 0:128], ps_a[:])
                    nc.scalar.copy(wvt[:, ep, 128:256], ps_b[:])

                # ---- output projection -> next layer's x^T ----
                xt_next = px.tile([128, EC, MT], bf16, tag="xt")
                for et in range(8):
                    wo_t = pwo.tile([128, EC, 128], bf16, tag="wo")
                    nc.sync.dma_start(
                        wo_t[:],
                        wot_p.ap()[wrow:wrow + E, 128 * et:128 * (et + 1)]
                        .rearrange("(c p) f -> p c f", p=128))
                    ps = psmm.tile([128, MT], f32, tag="mm")
                    for ec in range(EC):
                        nc.tensor.matmul(ps[:], wo_t[:, ec, :], wvt[:, ec, :],
                                         start=(ec == 0), stop=(ec == EC - 1))
                    nc.scalar.activation(xt_next[:, et, :], ps[:], AF.Identity,
                                         bias=bo_t[:, et:et + 1])
                xt = xt_next

            # ---- finalize: mean over layers, write output ----
            out_a = pscore.tile([128, 1024], f32, tag="sca")
            nc.scalar.mul(out_a[:], acc_a[:], 1.0 / L)
            nc.sync.dma_start(out_p[0:128, 0:1024], out_a[:])
            nc.sync.dma_start(out_p[0:128, 1024:2048], zeros[:])
            out_b = pscore.tile([128, 2048], f32, tag="scb")
            nc.scalar.mul(out_b[:], acc_b[:], 1.0 / L)
            nc.sync.dma_start(out_p[128:256, :], out_b[:])

    nc.compile()
    return nc


def _prep_in_maps(all_mentions, Wqkv, bqkv, Wo, bo):
    all_mentions = np.asarray(all_mentions, np.float32)
    Wqkv = np.asarray(Wqkv, np.float32)
    bqkv = np.asarray(bqkv, np.float32)
    Wo = np.asarray(Wo, np.float32)
    bo = np.asarray(bo, np.float32)

    Wq_scaled = Wqkv.copy()
    Wq_scaled[:, :E, :] *= SCALE
    wqkvt = np.ascontiguousarray(
        Wq_scaled.transpose(0, 2, 1)).reshape(L * E, 3 * E).astype(BF16)
    wot = np.ascontiguousarray(
        Wo.transpose(0, 2, 1)).reshape(L * E, E).astype(BF16)
    bqkv_s = bqkv.copy()
    bqkv_s[:, :E] *= SCALE
    bqkv_flat = np.ascontiguousarray(bqkv_s.reshape(-1), np.float32)
    bo_flat = np.ascontiguousarray(bo.reshape(-1), np.float32)
    ident = np.eye(128, dtype=BF16)

    p = np.arange(128)
    j1 = np.arange(1024)
    j2 = np.arange(2048)

    in_maps = []
    for c in range(NCORES):
        ta, tb = c, 15 - c
        rows = np.concatenate([np.arange(128 * ta, 128 * (ta + 1)),
                               np.arange(128 * tb, 128 * (tb + 1))])
        xt = np.ascontiguousarray(all_mentions[rows].T).astype(BF16)
        maska = np.where(j1[None, :] <= (128 * ta + p)[:, None],
                         np.float32(0.0), np.float32(NEG)).astype(np.float32)
        maskb = np.where(j2[None, :] <= (128 * tb + p)[:, None],
                         np.float32(0.0), np.float32(NEG)).astype(np.float32)
        in_maps.append({
            "xt": xt,
            "wqkvt": wqkvt,
            "wot": wot,
            "bqkv": bqkv_flat,
            "bo": bo_flat,
            "maska": maska,
            "maskb": maskb,
            "ident": ident,
        })
    return in_maps


class Runner:
    def __init__(self):
        self.nc = _build_nc()

    def run(self, in_maps, **kw):
        from concourse.bass_utils import run_bass_kernel_spmd
        return run_bass_kernel_spmd(self.nc, in_maps,
                                    core_ids=list(range(NCORES)), **kw)


def get_runner():
    global _RUNNER
    if _RUNNER is None:
        _RUNNER = Runner()
    return _RUNNER


def assemble_output(results):
    out = np.zeros((N, N), np.float32)
    for c in range(NCORES):
        o = np.asarray(results[c]["out"], np.float32)
        out[128 * c:128 * (c + 1), :1024] = o[0:128, :1024]
        out[128 * (15 - c):128 * (16 - c), :] = o[128:256, :]
    return out


def kernel(all_mentions, Wqkv, bqkv, Wo, bo):
    runner = get_runner()
    in_maps = _prep_in_maps(all_mentions, Wqkv, bqkv, Wo, bo)
    res = runner.run(in_maps)
    return assemble_output(res.results)
